# revision 33
# baseline (speedup 1.0000x reference)
"""Causal multi-head attention with RoPE on 8 Trainium2 NeuronCores.

Problem: B=2, L=2048, D_MODEL=1024, N_HEADS=16, D_K=64, theta=10000.
Sharding: data parallel on batch (2) x tensor parallel on heads (4 groups of
4 heads) = 8 cores. Each core computes its 4 heads' attention plus a partial
output projection; partials are summed on the host (Megatron row-parallel).

v3 design (baseline v2 was 218us in the cost model):
- All activations/weights in bf16; Q/K additionally stored as fp8e4 in
  [128, 2(T/B plane), L] layout so each scores matmul is one DoubleRow
  instruction per head per kv-tile (4x fewer PE cycles than fp32r K=32).
- Host packs xt/wq/wv/wo into [128, big] tensors so the whole input loads in
  ~13 large DMAs (the HWDGE has a ~625ns fixed cost per DMA).
- CH=256 q chunks; scoresT strips [kv128, 4h*256] ping-pong in PSUM; one exp
  ACT instruction per kv tile (exact causal slicing, bf16 out); the Pool
  engine zeroes the upper triangle of diagonal blocks in-place.
- AV swapped: out[q128, 65] per (head, qtile), exp'd scores stationary, V
  with an appended ones column (= softmax denominator) as the 65-wide moving
  operand, accumulated kv-tile-major so only the last 4 matmuls wait on exp.
- Normalize = DVE per-partition multiply by 1/denominator; the [q,128]
  head-pair block is transposed to ho[d,q] by the DMA engine's hardware
  transpose; output projection per l-tile with a 2-deep PSUM ring.
- A work queue interleaves next-chunk projection/V/RoPE and the deferred
  av/outproj units between score tiles so the in-order PE never blocks on a
  just-issued exp.
"""
import numpy as np
from contextlib import ExitStack

import concourse.bacc as bacc
import concourse.bass as bass
import concourse.mybir as mybir
import concourse.tile as tile
from concourse._compat import with_exitstack
from concourse.bass_utils import run_bass_kernel_spmd

F32 = mybir.dt.float32
BF16 = mybir.dt.bfloat16
FP8 = mybir.dt.float8e4

B, L, DM, NH, DK = 2, 2048, 1024, 16, 64
HPC = 4              # heads per core
THETA = 10000.0
CH = 256             # q chunk width
NCH = L // CH        # 8 chunks
NT = L // 128        # 16 kv tiles

_cache = {}
PHASE_RANGES = []   # (start_id, end_id, label) for trace attribution


def _track(nc, label, fn):
    def wrapped(*a, **k):
        s = nc.next_id()
        r = fn(*a, **k)
        PHASE_RANGES.append((s, nc.next_id(), f"{label}{a[:2]}"))
        return r
    return wrapped


@with_exitstack
def _attn_kernel(ctx: ExitStack, tc: tile.TileContext, outs, ins):
    nc = tc.nc
    xt, wq, wv, wo = ins["xt"], ins["wq"], ins["wv"], ins["wo"]
    cs, sn = ins["cs"], ins["sn"]
    out = outs["out"]
    AF = mybir.ActivationFunctionType
    DR = mybir.MatmulPerfMode.DoubleRow

    consts = ctx.enter_context(tc.tile_pool(name="consts", bufs=1))
    persist = ctx.enter_context(tc.tile_pool(name="persist", bufs=1))
    ps = ctx.enter_context(tc.tile_pool(name="ps", bufs=1, space="PSUM"))
    epool = ctx.enter_context(tc.tile_pool(name="epool", bufs=1))
    ropet = ctx.enter_context(tc.tile_pool(name="ropet", bufs=2))
    npool = ctx.enter_context(tc.tile_pool(name="npool", bufs=4))
    rpool = ctx.enter_context(tc.tile_pool(name="rpool", bufs=2))
    opool = ctx.enter_context(tc.tile_pool(name="opool", bufs=4))

    # ---- packed inputs: few large DMAs (HWDGE fixed cost dominates),
    # ordered so chunk 0's critical path loads first ----
    # xt: [128, (chunk c)(stripe d)(col j)] -> chunk c at [:, 2048c:2048c+2048]
    wq_sb = consts.tile([128, 4096], BF16)     # [(d)(512 cols: qT qB kT kB)]
    xt_sb = persist.tile([128, 8 * L], BF16)
    nc.sync.dma_start(wq_sb[:, 0:1024], wq[:, 0:1024])
    nc.sync.dma_start(xt_sb[:, 0:512], xt[:, 0:512])
    nc.sync.dma_start(wq_sb[:, 1024:2048], wq[:, 1024:2048])
    nc.sync.dma_start(xt_sb[:, 512:1024], xt[:, 512:1024])
    nc.sync.dma_start(wq_sb[:, 2048:3072], wq[:, 2048:3072])
    nc.sync.dma_start(xt_sb[:, 1024:2048], xt[:, 1024:2048])
    nc.sync.dma_start(wq_sb[:, 3072:4096], wq[:, 3072:4096])
    cs_sb = persist.tile([128, L], BF16)
    sn_sb = persist.tile([128, L], BF16)
    nc.sync.dma_start(cs_sb[:, 0:2 * CH], cs[:, 0:2 * CH])
    nc.sync.dma_start(sn_sb[:, 0:2 * CH], sn[:, 0:2 * CH])
    wv_sb = consts.tile([128, 2048], BF16)     # [(d)(256 cols)]
    nc.sync.dma_start(wv_sb, wv)
    nc.sync.dma_start(xt_sb[:, 2048:4096], xt[:, 2048:4096])
    wo_sb = consts.tile([128, 2048], BF16)     # [(j)(1024 cols)]
    nc.sync.dma_start(wo_sb, wo)
    nc.scalar.dma_start(cs_sb[:, 2 * CH:], cs[:, 2 * CH:])
    nc.scalar.dma_start(sn_sb[:, 2 * CH:], sn[:, 2 * CH:])

    def xs(c, d):
        return xt_sb[:, 2048 * c + 256 * d:2048 * c + 256 * d + 256]

    # persistent activations
    q8 = persist.tile([128, 2 * L], FP8, tag="q8")   # [4h*32, (T|B), L]
    k8 = persist.tile([128, 2 * L], FP8, tag="k8")
    q8p = q8[:].rearrange("p (two l) -> p two l", two=2)
    k8p = k8[:].rearrange("p (two l) -> p two l", two=2)
    v_sb, ho = [], []
    for t in range(NT):
        t_v = persist.tile([128, HPC * 65], BF16, tag=f"v{t}")
        v_sb.append(t_v)
    for j in range(2):
        t_ho = persist.tile([128, L], BF16, tag=f"ho{j}")
        ho.append(t_ho)

    def emit_xt_chunk(c):
        """Load xt chunk c (all 8 stripes) in one DMA on the ACT sequencer."""
        nc.scalar.dma_start(xt_sb[:, 2048 * c:2048 * (c + 1)],
                            xt[:, 2048 * c:2048 * (c + 1)])

    def emit_proj_qk_mms(c, part, grp):
        """One 4-matmul group (2 d-steps) of the Q (part=0) / K (part=1)
        projection for chunk c. Q accumulates in the pq bank, K in the op0
        bank so rope-Q (DVE) never blocks the K matmuls."""
        pq = state["pq"][part]
        for d in (2 * grp, 2 * grp + 1):
            for half in range(2):           # T, B
                csl = slice(512 * d + 256 * part + 128 * half,
                            512 * d + 256 * part + 128 * half + 128)
                nc.tensor.matmul(pq[:, 256 * half:256 * half + 256],
                                 wq_sb[:, csl], xs(c, d),
                                 start=(d == 0), stop=(d == 7))

    def emit_rope(c, part):
        """RoPE from the pq psum tile into q8/k8 fp8 planes."""
        pq = state["pq"][part]
        lsl = slice(CH * c, CH * (c + 1))
        dst = q8p if part == 0 else k8p
        cs_c, sn_c = cs_sb[:, lsl], sn_sb[:, lsl]
        pt, pb = pq[:, 0:256], pq[:, 256:512]
        t1 = ropet.tile([128, CH], F32, tag="t1")
        t2 = ropet.tile([128, CH], F32, tag="t2")
        nc.vector.tensor_mul(t1, pt, cs_c)
        nc.vector.tensor_mul(t2, pb, sn_c)
        nc.vector.tensor_sub(dst[:, 0, lsl], t1, t2)
        t3 = ropet.tile([128, CH], F32, tag="t1")
        t4 = ropet.tile([128, CH], F32, tag="t2")
        nc.vector.tensor_mul(t3, pb, cs_c)
        nc.vector.tensor_mul(t4, pt, sn_c)
        nc.vector.tensor_add(dst[:, 1, lsl], t3, t4)

    def emit_v_tile(tv):
        """V projection for kv tile tv + copy into v_sb with ones column."""
        vp = ps.tile([128, 260], F32, tag="av")
        v_ps = vp[:, 0:256]
        for d in range(8):
            xsl = xt_sb[:, 2048 * (tv // 2) + 256 * d + 128 * (tv % 2):
                        2048 * (tv // 2) + 256 * d + 128 * (tv % 2) + 128]
            nc.tensor.matmul(v_ps, xsl, wv_sb[:, 256 * d:256 * d + 256],
                             start=(d == 0), stop=(d == 7))
        vdst = v_sb[tv][:].rearrange("p (h x) -> p h x", x=65)[:, :, 0:64]
        vsrc = v_ps.rearrange("p (h x) -> p h x", x=64)
        nc.gpsimd.tensor_copy(vdst, vsrc)
        nc.gpsimd.memset(v_sb[tv][:, 64:HPC * 65:65], 1.0)

    def emit_scores_exp(c, t):
        """DoubleRow scores for kv tile t of chunk c + exp + diag mask."""
        qsl = slice(CH * c, CH * (c + 1))
        ksl = slice(128 * t, 128 * t + 128)
        strip = ps.tile([128, 4 * CH], F32, tag=f"sc{t % 2}")
        for h in range(HPC):
            hsl = slice(32 * h, 32 * h + 32)
            nc.tensor.matmul(strip[:, 256 * h:256 * h + 256],
                             k8p[hsl, :, ksl], q8p[hsl, :, qsl],
                             start=True, stop=True, perf_mode=DR,
                             tile_position=(32 * h, 0))
        expt = epool.tile([128, 4 * CH], BF16, tag=f"e{t % 16}")
        off = 128 if t == 2 * c + 1 else 0
        esrc = strip[:].rearrange("kv (h q) -> kv h q", q=CH)[:, :, off:]
        edst = expt[:].rearrange("kv (h q) -> kv h q", q=CH)[:, :, off:]
        nc.scalar.activation(edst, esrc, AF.Exp, scale=0.125)
        if t >= 2 * c:  # diagonal tile: zero the upper triangle of its block
            for h in range(HPC):
                blk = slice(256 * h + off, 256 * h + off + 128)
                nc.gpsimd.affine_select(expt[:, blk], expt[:, blk],
                                        pattern=[[1, 128]],
                                        compare_op=mybir.AluOpType.is_ge,
                                        fill=0.0, base=0, channel_multiplier=-1)
        return expt

    def emit_av_norm(c, qt, expts):
        """AV accumulation for q-block qt of chunk c, then normalize and
        transpose (DMA) into ho layout."""
        lt = 2 * c + qt
        av_ps = ps.tile([128, 260], F32, tag="av")
        ntile = 2 * c + qt + 1
        for t in range(ntile):  # t-major: only the last 4 mms wait on exp
            for h in range(HPC):
                nc.tensor.matmul(av_ps[:, 65 * h:65 * h + 65],
                                 expts[t][:, 256 * h + 128 * qt:256 * h + 128 * qt + 128],
                                 v_sb[t][:, 65 * h:65 * h + 65],
                                 start=(t == 0), stop=(t == ntile - 1))
        av_sb = rpool.tile([128, 260], F32, tag="avsb")
        nc.gpsimd.tensor_copy(av_sb, av_ps)
        av_n = []
        for j in range(2):
            t_n = npool.tile([128, 128], BF16, tag=f"n{j}")
            av_n.append(t_n)
        for h in range(HPC):
            nc.gpsimd.normalize_recip(av_n[h // 2][:, 64 * (h % 2):64 * (h % 2) + 64],
                                      av_sb[:, 65 * h:65 * h + 64],
                                      av_sb[:, 65 * h + 64:65 * h + 65])
        for j in range(2):
            nc.sync.dma_start_transpose(ho[j][:, 128 * lt:128 * lt + 128], av_n[j])

    def emit_outproj(lt):
        """Output projection + store for l-tile lt (ho rows already placed)."""
        last = lt == NT - 1
        o_sb = opool.tile([128, 1024], BF16, tag="o")
        for oc in range(2):
            op_ps = ps.tile([128, 512], F32, tag=f"op{oc}")
            for j in range(2):
                nc.tensor.matmul(op_ps, ho[j][:, 128 * lt:128 * lt + 128],
                                 wo_sb[:, 1024 * j + 512 * oc:1024 * j + 512 * oc + 512],
                                 start=(j == 0), stop=(j == 1))
            osl = slice(512 * oc, 512 * oc + 512)
            if last:
                # drain tail: fastest engines, store each half immediately
                if oc == 0:
                    nc.vector.tensor_copy(o_sb[:, osl], op_ps)
                else:
                    nc.scalar.copy(o_sb[:, osl], op_ps)
                nc.sync.dma_start(out[128 * lt:128 * lt + 128, osl], o_sb[:, osl])
            else:
                (nc.gpsimd if oc == 0 else nc.vector).tensor_copy(o_sb[:, osl], op_ps)
        if not last:
            nc.scalar.dma_start(out[128 * lt:128 * lt + 128, :], o_sb)

    def new_pq(part=0):
        # during the preamble K gets its own bank (op0, not yet used by
        # outproj) so rope-Q never blocks the K matmuls; afterwards both
        # parts share the pq bank to keep outproj decoupled
        tag = "op0" if (part == 1 and state.get("preamble")) else "pq"
        t_pq = ps.tile([128, 512], F32, tag=tag)
        state.setdefault("pq", [None, None])[part] = t_pq

    # ---- preamble: project chunks 0 and 1 ----
    state = {"preamble": True}
    for cc in range(2):
        new_pq(0)
        new_pq(1)
        for grp in range(4):
            emit_proj_qk_mms(cc, 0, grp)
            emit_proj_qk_mms(cc, 1, grp)
        emit_rope(cc, 0)
        emit_rope(cc, 1)
        emit_v_tile(2 * cc)
        emit_v_tile(2 * cc + 1)
        if cc == 0:
            emit_xt_chunk(2)

    state["preamble"] = False
    emit_scores_exp = _track(nc, "scores", emit_scores_exp)
    emit_av_norm = _track(nc, "av", emit_av_norm)
    emit_outproj = _track(nc, "outproj", emit_outproj)
    emit_v_tile = _track(nc, "vproj", emit_v_tile)
    emit_proj_qk_mms = _track(nc, "proj", emit_proj_qk_mms)
    emit_rope = _track(nc, "rope", emit_rope)

    # ---- main loop: per-tile scores/exp with a work queue of deferred PE
    # units (next-chunk projection, av batches, output projection) popped
    # between tiles so the PE never waits on a just-issued exp ----
    workq = []   # entries: (must_finish_this_chunk, fn)

    for c in range(NCH):
        if c + 3 < NCH:
            workq.append((True, lambda c2=c + 3: emit_xt_chunk(c2)))
        if c + 2 < NCH:
            def mk_qk(cc, part, grp, last):
                def f():
                    if grp == 0:
                        new_pq(part)
                    emit_proj_qk_mms(cc, part, grp)
                    if last:
                        emit_rope(cc, part)
                return f
            for part in range(2):
                for grp in range(4):
                    workq.append((True, mk_qk(c + 2, part, grp, grp == 3)))
            workq.append((True, lambda cc=c: emit_v_tile(2 * (cc + 2))))
            workq.append((True, lambda cc=c: emit_v_tile(2 * (cc + 2) + 1)))

        expts = []
        for t in range(2 * c + 2):
            expts.append(emit_scores_exp(c, t))
            if workq:
                workq.pop(0)[1]()
            if t == 2 * c:
                workq.insert(0, (False, lambda cc=c, ee=expts: emit_av_norm(cc, 0, ee)))
                workq.insert(1, (False, lambda cc=c: emit_outproj(2 * cc)))
            elif t == 2 * c + 1:
                workq.insert(0, (False, lambda cc=c, ee=expts: emit_av_norm(cc, 1, ee)))
                workq.insert(2, (False, lambda cc=c: emit_outproj(2 * cc + 1)))
        # next-chunk projection (and xt loads) must be in before its scores
        rest = []
        for must, fn in workq:
            if must:
                fn()
            else:
                rest.append((must, fn))
        workq = rest
    while workq:
        workq.pop(0)[1]()


def _build_nc():
    PHASE_RANGES.clear()
    nc = bacc.Bacc("TRN2", target_bir_lowering=False, debug=False,
                   enable_asserts=False, num_devices=8)
    ins = {
        "xt": nc.dram_tensor("xt", [128, 8 * L], BF16, kind="ExternalInput").ap(),
        "wq": nc.dram_tensor("wq", [128, 4096], BF16, kind="ExternalInput").ap(),
        "wv": nc.dram_tensor("wv", [128, 2048], BF16, kind="ExternalInput").ap(),
        "wo": nc.dram_tensor("wo", [128, 2048], BF16, kind="ExternalInput").ap(),
        "cs": nc.dram_tensor("cs", [128, L], BF16, kind="ExternalInput").ap(),
        "sn": nc.dram_tensor("sn", [128, L], BF16, kind="ExternalInput").ap(),
    }
    outs = {"out": nc.dram_tensor("out", [L, DM], BF16, kind="ExternalOutput").ap()}
    with tile.TileContext(nc) as tc:
        _attn_kernel(tc, outs, ins)
    nc.compile()
    return nc


def _host_shard(X, token_positions, Wqkv, Wout):
    """Build the 8 per-core input maps (bf16, packed layouts)."""
    import ml_dtypes
    bf = ml_dtypes.bfloat16
    X = np.asarray(X, dtype=np.float32)
    Wqkv = np.asarray(Wqkv, dtype=np.float32)
    Wout = np.asarray(Wout, dtype=np.float32)
    pos = np.asarray(token_positions)

    k = np.arange(DK // 2, dtype=np.float32)
    inv_freq = (np.float32(1.0) /
                np.power(np.float32(THETA), (np.float32(2.0) * k) / np.float32(DK)))
    ang = (pos.astype(np.float32)[:, None, :] *
           inv_freq.astype(np.float32)[None, :, None]).astype(np.float32)  # [B,32,L]
    cos = np.cos(ang).astype(np.float32)
    sin = np.sin(ang).astype(np.float32)
    cs_all = np.tile(cos, (1, HPC, 1)).astype(bf)  # [B, 128, L]
    sn_all = np.tile(sin, (1, HPC, 1)).astype(bf)

    in_maps = []
    for core in range(8):
        b, g = divmod(core, HPC)
        heads = [HPC * g + hh for hh in range(HPC)]
        q_top, q_bot, k_top, k_bot = [], [], [], []
        for h in heads:                      # psum rows: all-heads T, then B
            base = DK * h
            q_top += [base + 2 * kk for kk in range(DK // 2)]
            q_bot += [base + 2 * kk + 1 for kk in range(DK // 2)]
            k_top += [DM + base + 2 * kk for kk in range(DK // 2)]
            k_bot += [DM + base + 2 * kk + 1 for kk in range(DK // 2)]
        wq_c = np.ascontiguousarray(Wqkv[q_top + q_bot + k_top + k_bot, :].T)  # [1024, 512]
        wq_pk = np.ascontiguousarray(
            wq_c.reshape(8, 128, 512).transpose(1, 0, 2).reshape(128, 4096)).astype(bf)
        v_rows = [2 * DM + DK * h + j for h in heads for j in range(DK)]
        wv_c = Wqkv[v_rows, :].T                                          # [1024, 256]
        wv_pk = np.ascontiguousarray(
            wv_c.reshape(8, 128, 256).transpose(1, 0, 2).reshape(128, 2048)).astype(bf)
        wo_c = Wout[:, 256 * g:256 * (g + 1)].T                           # [256, 1024]
        wo_pk = np.ascontiguousarray(
            wo_c.reshape(2, 128, 1024).transpose(1, 0, 2).reshape(128, 2048)).astype(bf)
        xt_c = X[b].T                                                     # [1024, 2048]
        # [128, (c)(d)(j)] packing
        xt_pk = np.ascontiguousarray(
            xt_c.reshape(8, 128, 8, 256).transpose(1, 2, 0, 3).reshape(128, 8 * L)).astype(bf)
        in_maps.append({
            "xt": xt_pk,
            "wq": wq_pk,
            "wv": wv_pk,
            "wo": wo_pk,
            "cs": np.ascontiguousarray(cs_all[b]),
            "sn": np.ascontiguousarray(sn_all[b]),
        })
    return in_maps


def kernel(X, token_positions, Wqkv, Wout, _trace=False):
    if "nc" not in _cache:
        _cache["nc"] = _build_nc()
    nc = _cache["nc"]
    in_maps = _host_shard(X, token_positions, Wqkv, Wout)
    res = run_bass_kernel_spmd(nc, in_maps, list(range(8)), trace=_trace)
    _cache["last_results"] = res
    out = np.zeros((B, L, DM), dtype=np.float32)
    for core in range(8):
        out[core // HPC] += np.asarray(res.results[core]["out"], dtype=np.float32)
    return out


# revision 34
# speedup vs baseline: 1.0675x; 1.0675x over previous
"""Causal multi-head attention with RoPE on 8 Trainium2 NeuronCores.

Problem: B=2, L=2048, D_MODEL=1024, N_HEADS=16, D_K=64, theta=10000.
Sharding: data parallel on batch (2) x tensor parallel on heads (4 groups of
4 heads) = 8 cores. Each core computes its 4 heads' attention plus a partial
output projection; partials are summed on the host (Megatron row-parallel).

v3 design (baseline v2 was 218us in the cost model):
- All activations/weights in bf16; Q/K additionally stored as fp8e4 in
  [128, 2(T/B plane), L] layout so each scores matmul is one DoubleRow
  instruction per head per kv-tile (4x fewer PE cycles than fp32r K=32).
- Host packs xt/wq/wv/wo into [128, big] tensors so the whole input loads in
  ~13 large DMAs (the HWDGE has a ~625ns fixed cost per DMA).
- CH=256 q chunks; scoresT strips [kv128, 4h*256] ping-pong in PSUM; one exp
  ACT instruction per kv tile (exact causal slicing, bf16 out); the Pool
  engine zeroes the upper triangle of diagonal blocks in-place.
- AV swapped: out[q128, 65] per (head, qtile), exp'd scores stationary, V
  with an appended ones column (= softmax denominator) as the 65-wide moving
  operand, accumulated kv-tile-major so only the last 4 matmuls wait on exp.
- Normalize = DVE per-partition multiply by 1/denominator; the [q,128]
  head-pair block is transposed to ho[d,q] by the DMA engine's hardware
  transpose; output projection per l-tile with a 2-deep PSUM ring.
- A work queue interleaves next-chunk projection/V/RoPE and the deferred
  av/outproj units between score tiles so the in-order PE never blocks on a
  just-issued exp.
"""
import numpy as np
from contextlib import ExitStack

import concourse.bacc as bacc
import concourse.bass as bass
import concourse.mybir as mybir
import concourse.tile as tile
from concourse._compat import with_exitstack
from concourse.bass_utils import run_bass_kernel_spmd

F32 = mybir.dt.float32
BF16 = mybir.dt.bfloat16
FP8 = mybir.dt.float8e4

B, L, DM, NH, DK = 2, 2048, 1024, 16, 64
HPC = 4              # heads per core
THETA = 10000.0
CH = 256             # q chunk width
NCH = L // CH        # 8 chunks
NT = L // 128        # 16 kv tiles

_cache = {}
PHASE_RANGES = []   # (start_id, end_id, label) for trace attribution


def _track(nc, label, fn):
    def wrapped(*a, **k):
        s = nc.next_id()
        r = fn(*a, **k)
        PHASE_RANGES.append((s, nc.next_id(), f"{label}{a[:2]}"))
        return r
    return wrapped


@with_exitstack
def _attn_kernel(ctx: ExitStack, tc: tile.TileContext, outs, ins):
    nc = tc.nc
    xt, wq, wv, wo = ins["xt"], ins["wq"], ins["wv"], ins["wo"]
    cs, sn = ins["cs"], ins["sn"]
    out = outs["out"]
    AF = mybir.ActivationFunctionType
    DR = mybir.MatmulPerfMode.DoubleRow

    consts = ctx.enter_context(tc.tile_pool(name="consts", bufs=1))
    persist = ctx.enter_context(tc.tile_pool(name="persist", bufs=1))
    ps = ctx.enter_context(tc.tile_pool(name="ps", bufs=1, space="PSUM"))
    epool = ctx.enter_context(tc.tile_pool(name="epool", bufs=1))
    ropet = ctx.enter_context(tc.tile_pool(name="ropet", bufs=2))
    npool = ctx.enter_context(tc.tile_pool(name="npool", bufs=4))
    rpool = ctx.enter_context(tc.tile_pool(name="rpool", bufs=2))
    opool = ctx.enter_context(tc.tile_pool(name="opool", bufs=4))

    # ---- packed inputs: few large DMAs (HWDGE fixed cost dominates),
    # ordered so chunk 0's critical path loads first ----
    # xt: [128, (chunk c)(stripe d)(col j)] -> chunk c at [:, 2048c:2048c+2048]
    wq_sb = consts.tile([128, 4096], BF16)     # [(d)(512 cols: qT qB kT kB)]
    xt_sb = persist.tile([128, 8 * L], BF16)
    nc.sync.dma_start(wq_sb[:, 0:1024], wq[:, 0:1024])
    nc.sync.dma_start(xt_sb[:, 0:512], xt[:, 0:512])
    nc.sync.dma_start(wq_sb[:, 1024:2048], wq[:, 1024:2048])
    nc.sync.dma_start(xt_sb[:, 512:1024], xt[:, 512:1024])
    nc.sync.dma_start(wq_sb[:, 2048:3072], wq[:, 2048:3072])
    nc.sync.dma_start(xt_sb[:, 1024:2048], xt[:, 1024:2048])
    nc.sync.dma_start(wq_sb[:, 3072:4096], wq[:, 3072:4096])
    cs_sb = persist.tile([128, L], BF16)
    sn_sb = persist.tile([128, L], BF16)
    nc.sync.dma_start(cs_sb[:, 0:2 * CH], cs[:, 0:2 * CH])
    nc.sync.dma_start(sn_sb[:, 0:2 * CH], sn[:, 0:2 * CH])
    wv_sb = consts.tile([128, 2048], BF16)     # [(d)(256 cols)]
    nc.sync.dma_start(wv_sb, wv)
    nc.sync.dma_start(xt_sb[:, 2048:4096], xt[:, 2048:4096])
    wo_sb = consts.tile([128, 2048], BF16)     # [(j)(1024 cols)]
    nc.sync.dma_start(wo_sb, wo)
    nc.scalar.dma_start(cs_sb[:, 2 * CH:], cs[:, 2 * CH:])
    nc.scalar.dma_start(sn_sb[:, 2 * CH:], sn[:, 2 * CH:])

    def xs(c, d):
        return xt_sb[:, 2048 * c + 256 * d:2048 * c + 256 * d + 256]

    # persistent activations
    q8 = persist.tile([128, 2 * L], FP8, tag="q8")   # [4h*32, (T|B), L]
    k8 = persist.tile([128, 2 * L], FP8, tag="k8")
    q8p = q8[:].rearrange("p (two l) -> p two l", two=2)
    k8p = k8[:].rearrange("p (two l) -> p two l", two=2)
    v_sb, ho = [], []
    for t in range(NT):
        t_v = persist.tile([128, HPC * 65], BF16, tag=f"v{t}")
        v_sb.append(t_v)
    for j in range(2):
        t_ho = persist.tile([128, L], BF16, tag=f"ho{j}")
        ho.append(t_ho)

    def emit_xt_chunk(c):
        """Load xt chunk c (all 8 stripes) in one DMA on the ACT sequencer."""
        nc.scalar.dma_start(xt_sb[:, 2048 * c:2048 * (c + 1)],
                            xt[:, 2048 * c:2048 * (c + 1)])

    def emit_proj_qk_mms(c, part, grp):
        """One 4-matmul group (2 d-steps) of the Q (part=0) / K (part=1)
        projection for chunk c. Q accumulates in the pq bank, K in the op0
        bank so rope-Q (DVE) never blocks the K matmuls."""
        pq = state["pq"][part]
        for d in (2 * grp, 2 * grp + 1):
            for half in range(2):           # T, B
                csl = slice(512 * d + 256 * part + 128 * half,
                            512 * d + 256 * part + 128 * half + 128)
                nc.tensor.matmul(pq[:, 256 * half:256 * half + 256],
                                 wq_sb[:, csl], xs(c, d),
                                 start=(d == 0), stop=(d == 7))

    def emit_rope(c, part):
        """RoPE from the pq psum tile into q8/k8 fp8 planes."""
        pq = state["pq"][part]
        lsl = slice(CH * c, CH * (c + 1))
        dst = q8p if part == 0 else k8p
        cs_c, sn_c = cs_sb[:, lsl], sn_sb[:, lsl]
        pt, pb = pq[:, 0:256], pq[:, 256:512]
        t1 = ropet.tile([128, CH], F32, tag="t1")
        t2 = ropet.tile([128, CH], F32, tag="t2")
        nc.vector.tensor_mul(t1, pt, cs_c)
        nc.vector.tensor_mul(t2, pb, sn_c)
        nc.vector.tensor_sub(dst[:, 0, lsl], t1, t2)
        t3 = ropet.tile([128, CH], F32, tag="t1")
        t4 = ropet.tile([128, CH], F32, tag="t2")
        nc.vector.tensor_mul(t3, pb, cs_c)
        nc.vector.tensor_mul(t4, pt, sn_c)
        nc.vector.tensor_add(dst[:, 1, lsl], t3, t4)

    def emit_v_tile(tv):
        """V projection for kv tile tv + copy into v_sb with ones column."""
        vp = ps.tile([128, 260], F32, tag="av")
        v_ps = vp[:, 0:256]
        for d in range(8):
            xsl = xt_sb[:, 2048 * (tv // 2) + 256 * d + 128 * (tv % 2):
                        2048 * (tv // 2) + 256 * d + 128 * (tv % 2) + 128]
            nc.tensor.matmul(v_ps, xsl, wv_sb[:, 256 * d:256 * d + 256],
                             start=(d == 0), stop=(d == 7))
        vdst = v_sb[tv][:].rearrange("p (h x) -> p h x", x=65)[:, :, 0:64]
        vsrc = v_ps.rearrange("p (h x) -> p h x", x=64)
        nc.gpsimd.tensor_copy(vdst, vsrc)
        nc.gpsimd.memset(v_sb[tv][:, 64:HPC * 65:65], 1.0)

    def emit_scores_exp(c, t):
        """DoubleRow scores for kv tile t of chunk c + exp + diag mask."""
        qsl = slice(CH * c, CH * (c + 1))
        ksl = slice(128 * t, 128 * t + 128)
        strip = ps.tile([128, 4 * CH], F32, tag=f"sc{t % 2}")
        for h in range(HPC):
            hsl = slice(32 * h, 32 * h + 32)
            nc.tensor.matmul(strip[:, 256 * h:256 * h + 256],
                             k8p[hsl, :, ksl], q8p[hsl, :, qsl],
                             start=True, stop=True, perf_mode=DR,
                             tile_position=(32 * h, 0))
        expt = epool.tile([128, 4 * CH], BF16, tag=f"e{t % 16}")
        off = 128 if t == 2 * c + 1 else 0
        esrc = strip[:].rearrange("kv (h q) -> kv h q", q=CH)[:, :, off:]
        edst = expt[:].rearrange("kv (h q) -> kv h q", q=CH)[:, :, off:]
        nc.scalar.activation(edst, esrc, AF.Exp, scale=0.125)
        if t >= 2 * c:  # diagonal tile: zero the upper triangle of its block
            for h in range(HPC):
                blk = slice(256 * h + off, 256 * h + off + 128)
                nc.gpsimd.affine_select(expt[:, blk], expt[:, blk],
                                        pattern=[[1, 128]],
                                        compare_op=mybir.AluOpType.is_ge,
                                        fill=0.0, base=0, channel_multiplier=-1)
        return expt

    def emit_av_norm(c, qt, expts):
        """AV accumulation for q-block qt of chunk c, then normalize and
        transpose (DMA) into ho layout."""
        lt = 2 * c + qt
        av_ps = ps.tile([128, 260], F32, tag="av")
        ntile = 2 * c + qt + 1
        for t in range(ntile):  # t-major: only the last 4 mms wait on exp
            for h in range(HPC):
                nc.tensor.matmul(av_ps[:, 65 * h:65 * h + 65],
                                 expts[t][:, 256 * h + 128 * qt:256 * h + 128 * qt + 128],
                                 v_sb[t][:, 65 * h:65 * h + 65],
                                 start=(t == 0), stop=(t == ntile - 1))
        av_sb = rpool.tile([128, 260], F32, tag="avsb")
        nc.gpsimd.tensor_copy(av_sb, av_ps)
        av_n = []
        for j in range(2):
            t_n = npool.tile([128, 128], BF16, tag=f"n{j}")
            av_n.append(t_n)
        for h in range(HPC):
            nc.gpsimd.normalize_recip(av_n[h // 2][:, 64 * (h % 2):64 * (h % 2) + 64],
                                      av_sb[:, 65 * h:65 * h + 64],
                                      av_sb[:, 65 * h + 64:65 * h + 65])
        for j in range(2):
            nc.sync.dma_start_transpose(ho[j][:, 128 * lt:128 * lt + 128], av_n[j])

    def emit_outproj(lt):
        """Output projection + store for l-tile lt (ho rows already placed)."""
        last = lt == NT - 1
        o_sb = opool.tile([128, 1024], BF16, tag="o")
        for oc in range(2):
            op_ps = ps.tile([128, 512], F32, tag=f"op{oc}")
            for j in range(2):
                nc.tensor.matmul(op_ps, ho[j][:, 128 * lt:128 * lt + 128],
                                 wo_sb[:, 1024 * j + 512 * oc:1024 * j + 512 * oc + 512],
                                 start=(j == 0), stop=(j == 1))
            osl = slice(512 * oc, 512 * oc + 512)
            if last:
                # drain tail: fastest engines, store each half immediately
                if oc == 0:
                    nc.vector.tensor_copy(o_sb[:, osl], op_ps)
                else:
                    nc.scalar.copy(o_sb[:, osl], op_ps)
                nc.sync.dma_start(out[128 * lt:128 * lt + 128, osl], o_sb[:, osl])
            else:
                (nc.gpsimd if oc == 0 else nc.vector).tensor_copy(o_sb[:, osl], op_ps)
        if not last:
            nc.sync.dma_start(out[128 * lt:128 * lt + 128, :], o_sb)

    def new_pq(part=0):
        # during the preamble K gets its own bank (op0, not yet used by
        # outproj) so rope-Q never blocks the K matmuls; afterwards both
        # parts share the pq bank to keep outproj decoupled
        tag = "op0" if (part == 1 and state.get("preamble")) else "pq"
        t_pq = ps.tile([128, 512], F32, tag=tag)
        state.setdefault("pq", [None, None])[part] = t_pq

    # ---- preamble: project chunks 0 and 1 ----
    state = {"preamble": True}
    new_pq(0)
    new_pq(1)
    for grp in range(4):
        emit_proj_qk_mms(0, 0, grp)
        emit_proj_qk_mms(0, 1, grp)
    emit_rope(0, 0)
    emit_rope(0, 1)
    emit_v_tile(0)
    emit_v_tile(1)
    emit_xt_chunk(2)

    emit_scores_exp = _track(nc, "scores", emit_scores_exp)
    emit_av_norm = _track(nc, "av", emit_av_norm)
    emit_outproj = _track(nc, "outproj", emit_outproj)
    emit_v_tile = _track(nc, "vproj", emit_v_tile)
    emit_proj_qk_mms = _track(nc, "proj", emit_proj_qk_mms)
    emit_rope = _track(nc, "rope", emit_rope)

    # ---- main loop: per-tile scores/exp with a work queue of deferred PE
    # units (next-chunk projection, av batches, output projection) popped
    # between tiles so the PE never waits on a just-issued exp ----
    workq = []   # entries: (must_finish_this_chunk, fn)

    def mk_qk(cc, part, grp, last):
        def f():
            if grp == 0:
                new_pq(part)
            emit_proj_qk_mms(cc, part, grp)
            if last:
                emit_rope(cc, part)
        return f

    for part in range(2):
        for grp in range(4):
            workq.append((True, mk_qk(1, part, grp, grp == 3)))
    workq.append((True, lambda: emit_v_tile(2)))
    workq.append((True, lambda: emit_v_tile(3)))
    workq.append((True, lambda: state.update(preamble=False)))

    for c in range(NCH):
        if c + 3 < NCH:
            workq.append((True, lambda c2=c + 3: emit_xt_chunk(c2)))
        if c + 2 < NCH:
            for part in range(2):
                for grp in range(4):
                    workq.append((True, mk_qk(c + 2, part, grp, grp == 3)))
            workq.append((True, lambda cc=c: emit_v_tile(2 * (cc + 2))))
            workq.append((True, lambda cc=c: emit_v_tile(2 * (cc + 2) + 1)))

        expts = []
        for t in range(2 * c + 2):
            expts.append(emit_scores_exp(c, t))
            if workq:
                workq.pop(0)[1]()
            if t == 2 * c:
                workq.insert(0, (False, lambda cc=c, ee=expts: emit_av_norm(cc, 0, ee)))
                workq.insert(1, (False, lambda cc=c: emit_outproj(2 * cc)))
            elif t == 2 * c + 1:
                workq.insert(0, (False, lambda cc=c, ee=expts: emit_av_norm(cc, 1, ee)))
                workq.insert(2, (False, lambda cc=c: emit_outproj(2 * cc + 1)))
        # next-chunk projection (and xt loads) must be in before its scores
        rest = []
        for must, fn in workq:
            if must:
                fn()
            else:
                rest.append((must, fn))
        workq = rest
    while workq:
        workq.pop(0)[1]()


def _build_nc():
    PHASE_RANGES.clear()
    nc = bacc.Bacc("TRN2", target_bir_lowering=False, debug=False,
                   enable_asserts=False, num_devices=8)
    ins = {
        "xt": nc.dram_tensor("xt", [128, 8 * L], BF16, kind="ExternalInput").ap(),
        "wq": nc.dram_tensor("wq", [128, 4096], BF16, kind="ExternalInput").ap(),
        "wv": nc.dram_tensor("wv", [128, 2048], BF16, kind="ExternalInput").ap(),
        "wo": nc.dram_tensor("wo", [128, 2048], BF16, kind="ExternalInput").ap(),
        "cs": nc.dram_tensor("cs", [128, L], BF16, kind="ExternalInput").ap(),
        "sn": nc.dram_tensor("sn", [128, L], BF16, kind="ExternalInput").ap(),
    }
    outs = {"out": nc.dram_tensor("out", [L, DM], BF16, kind="ExternalOutput").ap()}
    with tile.TileContext(nc) as tc:
        _attn_kernel(tc, outs, ins)
    nc.compile()
    return nc


def _host_shard(X, token_positions, Wqkv, Wout):
    """Build the 8 per-core input maps (bf16, packed layouts)."""
    import ml_dtypes
    bf = ml_dtypes.bfloat16
    X = np.asarray(X, dtype=np.float32)
    Wqkv = np.asarray(Wqkv, dtype=np.float32)
    Wout = np.asarray(Wout, dtype=np.float32)
    pos = np.asarray(token_positions)

    k = np.arange(DK // 2, dtype=np.float32)
    inv_freq = (np.float32(1.0) /
                np.power(np.float32(THETA), (np.float32(2.0) * k) / np.float32(DK)))
    ang = (pos.astype(np.float32)[:, None, :] *
           inv_freq.astype(np.float32)[None, :, None]).astype(np.float32)  # [B,32,L]
    cos = np.cos(ang).astype(np.float32)
    sin = np.sin(ang).astype(np.float32)
    cs_all = np.tile(cos, (1, HPC, 1)).astype(bf)  # [B, 128, L]
    sn_all = np.tile(sin, (1, HPC, 1)).astype(bf)

    in_maps = []
    for core in range(8):
        b, g = divmod(core, HPC)
        heads = [HPC * g + hh for hh in range(HPC)]
        q_top, q_bot, k_top, k_bot = [], [], [], []
        for h in heads:                      # psum rows: all-heads T, then B
            base = DK * h
            q_top += [base + 2 * kk for kk in range(DK // 2)]
            q_bot += [base + 2 * kk + 1 for kk in range(DK // 2)]
            k_top += [DM + base + 2 * kk for kk in range(DK // 2)]
            k_bot += [DM + base + 2 * kk + 1 for kk in range(DK // 2)]
        wq_c = np.ascontiguousarray(Wqkv[q_top + q_bot + k_top + k_bot, :].T)  # [1024, 512]
        wq_pk = np.ascontiguousarray(
            wq_c.reshape(8, 128, 512).transpose(1, 0, 2).reshape(128, 4096)).astype(bf)
        v_rows = [2 * DM + DK * h + j for h in heads for j in range(DK)]
        wv_c = Wqkv[v_rows, :].T                                          # [1024, 256]
        wv_pk = np.ascontiguousarray(
            wv_c.reshape(8, 128, 256).transpose(1, 0, 2).reshape(128, 2048)).astype(bf)
        wo_c = Wout[:, 256 * g:256 * (g + 1)].T                           # [256, 1024]
        wo_pk = np.ascontiguousarray(
            wo_c.reshape(2, 128, 1024).transpose(1, 0, 2).reshape(128, 2048)).astype(bf)
        xt_c = X[b].T                                                     # [1024, 2048]
        # [128, (c)(d)(j)] packing
        xt_pk = np.ascontiguousarray(
            xt_c.reshape(8, 128, 8, 256).transpose(1, 2, 0, 3).reshape(128, 8 * L)).astype(bf)
        in_maps.append({
            "xt": xt_pk,
            "wq": wq_pk,
            "wv": wv_pk,
            "wo": wo_pk,
            "cs": np.ascontiguousarray(cs_all[b]),
            "sn": np.ascontiguousarray(sn_all[b]),
        })
    return in_maps


def kernel(X, token_positions, Wqkv, Wout, _trace=False):
    if "nc" not in _cache:
        _cache["nc"] = _build_nc()
    nc = _cache["nc"]
    in_maps = _host_shard(X, token_positions, Wqkv, Wout)
    res = run_bass_kernel_spmd(nc, in_maps, list(range(8)), trace=_trace)
    _cache["last_results"] = res
    out = np.zeros((B, L, DM), dtype=np.float32)
    for core in range(8):
        out[core // HPC] += np.asarray(res.results[core]["out"], dtype=np.float32)
    return out
